# revision 24
# baseline (speedup 1.0000x reference)
"""Causal multi-head attention (B=2, S=2048, D=1024, H=16) on 8 Trainium2
NeuronCores, tensor-parallel over heads: core c owns heads 2c and 2c+1
(a 128-wide slice of the QKV output dim / Wo input dim).

All matmuls float32r (TF32-like, full rate at N>=256). Key perf constraints
discovered on HW:
  - PE_HAM only counts full-array (K=128) matmuls as "busy"; K=64 streams
    run the PE at 1.2 GHz. So scores use K=128 with per-head zero-padded
    QT operands instead of K=64 contractions.
  - Mixing K=64 and K=128 matmuls breaks LDWEIGHTS pipelining (~2x); a
    uniform K=128 stream (M may vary) runs at ~227ns per N=512 matmul.
  - DVE reciprocal costs ~6.4 cyc per free-element per lane, so softmax
    denominators are DMA-reshaped to a [128, 4] partition-parallel layout,
    reciprocal'd there (~180ns), and DMA'd back for a K=1 broadcast matmul.

Dataflow per core:
  x --PE-transpose--> xT (per 512-row s-tile)
  QT0/QT1 (zero-padded per head), KT, VT = W.T @ xT  [c, 4096] layout
  per (batch, 512-q-tile, head): for k-blocks in groups of 2:
     S^T[k,q] = kT_blk.T @ QTh_tile   (K=128, N=512)
     P^T = exp(0.125*(S^T + tri_mask))   one ACT op per group, f32r out
     ctx^T[65,512] += V1_blk.T @ P^T[j0:]  (V1 = [V | 1]: row 64 = denom)
  denominators: DMA-> [128,4] -> DVE reciprocal -> DMA -> [1,512] row ->
     K=1 ones matmul broadcast -> DVE multiply (normalize ctx).
  out_partial = ctx^T.T @ WoT per 128-row s-block; PSUM->SBUF->DRAM.

Host side shards weights across cores and sums the 8 partial outputs + bias.
"""

import numpy as np

import concourse.mybir as mybir
from concourse import bacc
from concourse.bass_utils import run_bass_kernel_spmd
from concourse.tile import TileContext

B, S, D, H = 2, 2048, 1024, 16
HD = D // H              # 64
BS = B * S               # 4096
NCORES = 8
CP = 128                 # c-dim per core (2 heads x 64)
ST = 512                 # s-tile width for projections
NST = BS // ST           # 8
QTW = 512                # q-tile width for attention
NQT = S // QTW           # 4 per batch
KO = D // 128            # 8 d_in blocks
NKB = S // 128           # 16 k-blocks per batch
GRP = 2                  # k-blocks per exp group (2 PSUM banks)
NEG = -1.0e30

FP32 = mybir.dt.float32
FP32R = mybir.dt.float32r
AF = mybir.ActivationFunctionType
OP = mybir.AluOpType

_CACHE = {}


def _build():
    nc = bacc.Bacc("TRN2", target_bir_lowering=False, debug=False, num_devices=NCORES)

    x_d = nc.dram_tensor("x_in", [BS, D], FP32R, kind="ExternalInput")
    wq_d = nc.dram_tensor("wq_in", [CP, D], FP32R, kind="ExternalInput")
    wk_d = nc.dram_tensor("wk_in", [CP, D], FP32R, kind="ExternalInput")
    wv_d = nc.dram_tensor("wv_in", [CP, D], FP32R, kind="ExternalInput")
    wo_d = nc.dram_tensor("wo_in", [D, CP], FP32R, kind="ExternalInput")
    id_d = nc.dram_tensor("ident_in", [128, 128], FP32R, kind="ExternalInput")
    tri_d = nc.dram_tensor("tri_in", [128, 128], FP32, kind="ExternalInput")
    one_d = nc.dram_tensor("ones_in", [128, 128], FP32R, kind="ExternalInput")
    zero_d = nc.dram_tensor("zeros_in", [64, BS], FP32R, kind="ExternalInput")
    out_d = nc.dram_tensor("out_p", [BS, D], FP32, kind="ExternalOutput")

    with TileContext(nc) as tc:
        with (
            tc.tile_pool(name="const", bufs=1) as constp,
            tc.tile_pool(name="big", bufs=1) as bigp,
            tc.tile_pool(name="xload", bufs=3) as xloadp,
            tc.tile_pool(name="xt", bufs=1) as xtp,
            tc.tile_pool(name="pt", bufs=2) as ptp,
            tc.tile_pool(name="work", bufs=3) as workp,
            tc.tile_pool(name="cdp", bufs=4) as cdp,
            tc.tile_pool(name="dram", bufs=8, space="DRAM") as dramp,
            tc.tile_pool(name="psS", bufs=2, space="PSUM") as psS,
            tc.tile_pool(name="psacc", bufs=4, space="PSUM") as psacc,
        ):
            # ---- constants -------------------------------------------------
            identt = constp.tile([128, 128], FP32R, tag="ident")
            nc.sync.dma_start(identt[:], id_d.ap())
            trit = constp.tile([128, 128], FP32, tag="tri")
            nc.sync.dma_start(trit[:], tri_d.ap())
            onest = constp.tile([128, 128], FP32R, tag="ones")
            nc.sync.dma_start(onest[:], one_d.ap())

            # ---- phase 0: weight slices, transposed on PE ------------------
            wqT = constp.tile([128, KO, 128], FP32R, tag="wqT")
            wkT = constp.tile([128, KO, 128], FP32R, tag="wkT")
            wvT = constp.tile([128, KO, 128], FP32R, tag="wvT")
            woT = constp.tile([128, KO, 128], FP32R, tag="woT")

            for dram, dst, natural in (
                (wq_d, wqT, True),
                (wk_d, wkT, True),
                (wv_d, wvT, True),
                (wo_d, woT, False),
            ):
                wl = xloadp.tile([128, KO, 128], FP32R, tag="wload")
                if natural:
                    nc.sync.dma_start(
                        wl[:], dram.ap().rearrange("c (ko p) -> c ko p", p=128)
                    )
                else:
                    nc.sync.dma_start(
                        wl[:], dram.ap().rearrange("(ko p) c -> p ko c", p=128)
                    )
                for g in range(2):
                    pst = psS.tile([128, 512], FP32R, tag="psS")
                    for j in range(4):
                        db = g * 4 + j
                        nc.tensor.transpose(
                            pst[:, j * 128 : (j + 1) * 128], wl[:, db, :], identt[:]
                        )
                    nc.vector.tensor_copy(
                        dst[:, g * 4 : (g + 1) * 4, :].rearrange("p a b -> p (a b)"),
                        pst[:],
                    )

            # ---- phase 1: x transpose + QKV projections + V1 assembly ------
            qT0 = bigp.tile([128, NST, ST], FP32R, tag="qT0")  # h0 rows, 64:128 zero
            qT1 = bigp.tile([128, NST, ST], FP32R, tag="qT1")  # h1 rows, 0:64 zero
            kT = bigp.tile([128, NST, ST], FP32R, tag="kT")
            vT = bigp.tile([128, NST, ST], FP32R, tag="vT")
            v1 = bigp.tile([128, B, 2, NKB, 65], FP32R, tag="v1")
            # ones column via DVE (a strided 4B-element DMA is pathological)
            nc.vector.tensor_copy(
                v1[:, :, :, :, 64],
                onest[:, 0 : B * 2 * NKB].rearrange("p (b h k) -> p b h k", b=B, h=2),
            )
            nc.sync.dma_start(
                qT0[64:128, :, :].rearrange("p a b -> p (a b)"), zero_d.ap()
            )
            nc.sync.dma_start(
                qT1[0:64, :, :].rearrange("p a b -> p (a b)"), zero_d.ap()
            )

            def v1_assemble(st):
                # V1 (k-major V + ones) for s-tile st's 4 k-blocks
                b = st // 4
                for hp in range(2):
                    pst = psS.tile([128, 512], FP32R, tag="psS", name=f"v1t_{st}_{hp}")
                    for j in range(4):
                        kb = (st % 4) * 4 + j
                        sti, off = divmod(b * S + kb * 128, ST)
                        nc.tensor.transpose(
                            pst[:, j * 64 : (j + 1) * 64],
                            vT[64 * hp : 64 * hp + 64, sti, off : off + 128],
                            identt[64 * hp : 64 * hp + 64, 64 * hp : 64 * hp + 64],
                        )
                    nc.vector.tensor_copy(
                        v1[:, b, hp, (st % 4) * 4 : (st % 4) * 4 + 4, 0:64],
                        pst[:, 0:256].rearrange("p (a c) -> p a c", a=4),
                    )

            for st in range(NST):
                xls = []
                for h in range(2):
                    xl = xloadp.tile([128, 2, D], FP32R, tag="xl")
                    r0 = st * ST + h * 256
                    nc.sync.dma_start(
                        xl[:],
                        x_d.ap()[r0 : r0 + 256, :].rearrange(
                            "(sb p) d -> p sb d", p=128
                        ),
                    )
                    xls.append(xl)
                xt = xtp.tile([128, KO, ST], FP32R, tag="xt")
                psq = psacc.tile([128, ST], FP32, tag="acc")
                psk = psacc.tile([128, ST], FP32, tag="acc")
                psv = psacc.tile([128, ST], FP32, tag="acc")
                # Interleave transpose batches with projection matmuls (one db
                # of lag) so full-array matmuls keep PE_HAM's busy monitor fed.
                for db in range(KO + 1):
                    if db < KO:
                        pst = psS.tile([128, 512], FP32R, tag="psS")
                        for sb in range(4):
                            nc.tensor.transpose(
                                pst[:, sb * 128 : (sb + 1) * 128],
                                xls[sb // 2][:, sb % 2, db * 128 : (db + 1) * 128],
                                identt[:],
                            )
                        nc.vector.tensor_copy(xt[:, db, :], pst[:])
                    if db == 1 and st > 0:
                        v1_assemble(st - 1)
                    if db > 0:
                        pdb = db - 1
                        first, last = pdb == 0, pdb == KO - 1
                        nc.tensor.matmul(
                            psq[:], wqT[:, pdb, :], xt[:, pdb, :], start=first, stop=last
                        )
                        nc.tensor.matmul(
                            psk[:], wkT[:, pdb, :], xt[:, pdb, :], start=first, stop=last
                        )
                        nc.tensor.matmul(
                            psv[:], wvT[:, pdb, :], xt[:, pdb, :], start=first, stop=last
                        )
                nc.vector.tensor_copy(qT0[0:64, st, :], psq[0:64, :])
                nc.vector.tensor_copy(qT1[64:128, st, :], psq[64:128, :])
                nc.vector.tensor_copy(kT[:, st, :], psk[:])
                nc.vector.tensor_copy(vT[:, st, :], psv[:])
            v1_assemble(NST - 1)

            # ---- phase 3+4: attention + output projection per batch --------
            # Software pipelining: scores of group g+1 are issued before the
            # AV matmuls of group g (PE never waits on the ACT exp), and the
            # softmax-normalization tail of q-tile qt is issued during q-tile
            # qt+1 so its DMA round-trip latency is hidden.
            qTs = (qT0, qT1)
            rpads = {}
            deferred = []  # closures: division tails + outproj, drained lazily
            for b in range(B):
                ctx = bigp.tile([128, S], FP32R, tag=f"ctx{b}")

                def div_head(qt, hp, cd, b=b, ctx=ctx):
                    # normalization tail: needs cd (ctx+den copy) + its rpad
                    q0 = qt * QTW
                    rrow = workp.tile(
                        [1, QTW], FP32R, tag="rrow", name=f"rrow_{b}_{qt}_{hp}"
                    )
                    nc.gpsimd.dma_start(rrow[:], rpads[(b, qt, hp)][:])
                    rbc = psacc.tile(
                        [128, QTW], FP32, tag="acc", name=f"rbc_{b}_{qt}_{hp}"
                    )
                    nc.tensor.matmul(
                        rbc[:], onest[0:1, :], rrow[:], start=True, stop=True
                    )
                    nc.vector.tensor_tensor(
                        ctx[64 * hp : 64 * hp + 64, q0 : q0 + QTW],
                        cd[0:64, :],
                        rbc[0:64, :],
                        OP.mult,
                    )

                def outproj(sb, ot, b=b, ctx=ctx):
                    po = psacc.tile(
                        [128, 512], FP32, tag="acc", name=f"po_{b}_{sb}_{ot}"
                    )
                    nc.tensor.matmul(
                        po[:],
                        ctx[:, sb * 128 : (sb + 1) * 128],
                        woT[:, ot * 4 : (ot + 1) * 4, :].rearrange("p a b -> p (a b)"),
                        start=True,
                        stop=True,
                    )
                    ost = workp.tile([128, 512], FP32, tag="ost")
                    if (sb + ot) % 2 == 0:
                        nc.scalar.activation(ost[:], po[:], AF.Copy)
                    else:
                        nc.vector.tensor_copy(ost[:], po[:])
                    r0 = b * S + sb * 128
                    nc.sync.dma_start(
                        out_d.ap()[r0 : r0 + 128, ot * 512 : (ot + 1) * 512],
                        ost[:],
                    )

                for qt in range(NQT):
                    q0 = qt * QTW
                    stq = (b * S + q0) // ST
                    av0 = psacc.tile([65, QTW], FP32, tag="acc", name=f"av0_{b}_{qt}")
                    av1 = psacc.tile([65, QTW], FP32, tag="acc", name=f"av1_{b}_{qt}")
                    av = (av0, av1)
                    nkb = (q0 + QTW) // 128
                    groups = [
                        (hp, list(range(g, min(g + GRP, nkb))))
                        for hp in range(2)
                        for g in range(0, nkb, GRP)
                    ]
                    prev = None  # (hp, kbs, pt)
                    for gi, grp_item in enumerate(groups + [None]):
                        hp, kbs = grp_item if grp_item is not None else (None, None)
                        if gi < len(groups):
                            pss = psS.tile(
                                [128, GRP, QTW], FP32, tag="psS",
                                name=f"pss_{b}_{qt}_{hp}_{kbs[0]}",
                            )
                            for j, kb in enumerate(kbs):
                                k0 = kb * 128
                                stk, offk = divmod(b * S + k0, ST)
                                nc.tensor.matmul(
                                    pss[:, j, :],
                                    kT[:, stk, offk : offk + 128],
                                    qTs[hp][:, stq, :],
                                    start=True,
                                    stop=True,
                                )
                                if k0 >= q0:
                                    j0 = k0 - q0
                                    nc.vector.tensor_tensor(
                                        pss[:, j, j0 : j0 + 128],
                                        pss[:, j, j0 : j0 + 128],
                                        trit[:],
                                        OP.add,
                                    )
                            pt = ptp.tile([128, GRP, QTW], FP32R, tag="pt")
                            nc.scalar.activation(
                                pt[:, :, :].rearrange("p a b -> p (a b)"),
                                pss[:, :, :].rearrange("p a b -> p (a b)"),
                                AF.Exp,
                                scale=0.125,
                            )
                        # AV of the previous group (pipelined behind scores)
                        if prev is not None:
                            phl, pkbs, ppt = prev
                            for j, kb in enumerate(pkbs):
                                j0 = max(0, kb * 128 - q0)
                                nc.tensor.matmul(
                                    av[phl][:, j0:QTW],
                                    v1[:, b, phl, kb, :],
                                    ppt[:, j, j0:QTW],
                                    start=(kb == 0),
                                    stop=(kb == nkb - 1),
                                )
                        prev = (hp, kbs, pt) if gi < len(groups) else None
                        # drain deferred work (qt-1 tails) spread across groups
                        if gi >= 1 and deferred:
                            n_emit = 2 if len(deferred) > len(groups) - gi else 1
                            for _ in range(n_emit):
                                if deferred:
                                    deferred.pop(0)()
                    # start this q-tile's normalization chains (latency hidden)
                    for hp in range(2):
                        cd = cdp.tile(
                            [65, QTW], FP32R, tag="cd", name=f"cd_{b}_{qt}_{hp}"
                        )
                        nc.scalar.activation(cd[:], av[hp][:], AF.Copy)
                        dpad = dramp.tile([1, QTW], FP32, tag="dpad")
                        nc.sync.dma_start(dpad[:], cd.bitcast(FP32)[64:65, :])
                        denT = workp.tile([128, 4], FP32, tag="denT")
                        nc.sync.dma_start(
                            denT[:], dpad.rearrange("o (p j) -> p (o j)", p=128)
                        )
                        recT = workp.tile([128, 4], FP32, tag="recT")
                        nc.vector.reciprocal(recT[:], denT[:])
                        rpad = dramp.tile(
                            [1, QTW], FP32, tag="rpad", name=f"rpad_{b}_{qt}_{hp}"
                        )
                        nc.sync.dma_start(
                            rpad.rearrange("o (p j) -> p (o j)", p=128), recT[:]
                        )
                        rpads[(b, qt, hp)] = rpad
                        deferred.append(
                            lambda qt=qt, hp=hp, cd=cd, fn=div_head: fn(qt, hp, cd)
                        )
                    for sb in range(4 * qt, 4 * qt + 4):
                        for ot in range(2):
                            deferred.append(lambda sb=sb, ot=ot, fn=outproj: fn(sb, ot))
            for fn in deferred:
                fn()
    nc.compile()
    return nc


def _get_nc():
    if "nc" not in _CACHE:
        _CACHE["nc"] = _build()
    return _CACHE["nc"]


def _consts():
    ident = np.eye(128, dtype=np.float32)
    p = np.arange(128)
    tri = np.where(p[:, None] <= p[None, :], 0.0, NEG).astype(np.float32)
    ones = np.ones((128, 128), dtype=np.float32)
    zeros = np.zeros((64, BS), dtype=np.float32)
    return ident, tri, ones, zeros


def make_in_maps(inputs):
    x = np.ascontiguousarray(np.asarray(inputs["x"], dtype=np.float32)).reshape(BS, D)
    Wq = np.asarray(inputs["Wq"], dtype=np.float32)
    Wk = np.asarray(inputs["Wk"], dtype=np.float32)
    Wv = np.asarray(inputs["Wv"], dtype=np.float32)
    Wo = np.asarray(inputs["Wo"], dtype=np.float32)

    ident, tri, ones, zeros = _consts()
    in_maps = []
    for c in range(NCORES):
        sl = slice(c * CP, (c + 1) * CP)
        in_maps.append(
            {
                "x_in": x,
                "wq_in": np.ascontiguousarray(Wq[sl]),
                "wk_in": np.ascontiguousarray(Wk[sl]),
                "wv_in": np.ascontiguousarray(Wv[sl]),
                "wo_in": np.ascontiguousarray(Wo[:, sl]),
                "ident_in": ident,
                "tri_in": tri,
                "ones_in": ones,
                "zeros_in": zeros,
            }
        )
    return in_maps


def reduce_outputs(results, bo):
    acc = np.zeros((BS, D), dtype=np.float64)
    for r in results:
        acc += r["out_p"]
    acc += np.asarray(bo, dtype=np.float64)
    return acc.astype(np.float32).reshape(B, S, D)


def kernel(**inputs):
    bo = np.asarray(inputs["bo"], dtype=np.float32)
    in_maps = make_in_maps(inputs)
    nc = _get_nc()
    res = run_bass_kernel_spmd(nc, in_maps, core_ids=list(range(NCORES)))
    return reduce_outputs(res.results, bo)


# revision 25
# speedup vs baseline: 1.0948x; 1.0948x over previous
"""Causal multi-head attention (B=2, S=2048, D=1024, H=16) on 8 Trainium2
NeuronCores, tensor-parallel over heads: core c owns heads 2c and 2c+1
(a 128-wide slice of the QKV output dim / Wo input dim).

All matmuls float32r (TF32-like, full rate at N>=256). Key perf constraints
discovered on HW:
  - PE_HAM only counts full-array (K=128) matmuls as "busy"; K=64 streams
    run the PE at 1.2 GHz. So scores use K=128 with per-head zero-padded
    QT operands instead of K=64 contractions.
  - Mixing K=64 and K=128 matmuls breaks LDWEIGHTS pipelining (~2x); a
    uniform K=128 stream (M may vary) runs at ~227ns per N=512 matmul.
  - DVE reciprocal costs ~6.4 cyc per free-element per lane, so softmax
    denominators are DMA-reshaped to a [128, 4] partition-parallel layout,
    reciprocal'd there (~180ns), and DMA'd back for a K=1 broadcast matmul.

Dataflow per core:
  x --PE-transpose--> xT (per 512-row s-tile)
  QT0/QT1 (zero-padded per head), KT, VT = W.T @ xT  [c, 4096] layout
  per (batch, 512-q-tile, head): for k-blocks in groups of 2:
     S^T[k,q] = kT_blk.T @ QTh_tile   (K=128, N=512)
     P^T = exp(0.125*(S^T + tri_mask))   one ACT op per group, f32r out
     ctx^T[65,512] += V1_blk.T @ P^T[j0:]  (V1 = [V | 1]: row 64 = denom)
  denominators: DMA-> [128,4] -> DVE reciprocal -> DMA -> [1,512] row ->
     K=1 ones matmul broadcast -> DVE multiply (normalize ctx).
  out_partial = ctx^T.T @ WoT per 128-row s-block; PSUM->SBUF->DRAM.

Host side shards weights across cores and sums the 8 partial outputs + bias.
"""

import numpy as np

import concourse.mybir as mybir
from concourse import bacc
from concourse.bass_utils import run_bass_kernel_spmd
from concourse.tile import TileContext

B, S, D, H = 2, 2048, 1024, 16
HD = D // H              # 64
BS = B * S               # 4096
NCORES = 8
CP = 128                 # c-dim per core (2 heads x 64)
ST = 512                 # s-tile width for projections
NST = BS // ST           # 8
QTW = 512                # q-tile width for attention
NQT = S // QTW           # 4 per batch
KO = D // 128            # 8 d_in blocks
NKB = S // 128           # 16 k-blocks per batch
GRP = 2                  # k-blocks per exp group (2 PSUM banks)
NEG = -1.0e30

FP32 = mybir.dt.float32
FP32R = mybir.dt.float32r
AF = mybir.ActivationFunctionType
OP = mybir.AluOpType

_CACHE = {}


def _build():
    nc = bacc.Bacc("TRN2", target_bir_lowering=False, debug=False, num_devices=NCORES)

    x_d = nc.dram_tensor("x_in", [BS, D], FP32R, kind="ExternalInput")
    wq_d = nc.dram_tensor("wq_in", [CP, D], FP32R, kind="ExternalInput")
    wk_d = nc.dram_tensor("wk_in", [CP, D], FP32R, kind="ExternalInput")
    wv_d = nc.dram_tensor("wv_in", [CP, D], FP32R, kind="ExternalInput")
    wo_d = nc.dram_tensor("wo_in", [D, CP], FP32R, kind="ExternalInput")
    id_d = nc.dram_tensor("ident_in", [128, 128], FP32R, kind="ExternalInput")
    tri_d = nc.dram_tensor("tri_in", [128, 128], FP32, kind="ExternalInput")
    one_d = nc.dram_tensor("ones_in", [128, 128], FP32R, kind="ExternalInput")
    zero_d = nc.dram_tensor("zeros_in", [64, BS], FP32R, kind="ExternalInput")
    out_d = nc.dram_tensor("out_p", [BS, D], FP32, kind="ExternalOutput")

    with TileContext(nc) as tc:
        with (
            tc.tile_pool(name="const", bufs=1) as constp,
            tc.tile_pool(name="big", bufs=1) as bigp,
            tc.tile_pool(name="xload", bufs=3) as xloadp,
            tc.tile_pool(name="xt", bufs=1) as xtp,
            tc.tile_pool(name="pt", bufs=2) as ptp,
            tc.tile_pool(name="work", bufs=3) as workp,
            tc.tile_pool(name="cdp", bufs=4) as cdp,
            tc.tile_pool(name="dram", bufs=8, space="DRAM") as dramp,
            tc.tile_pool(name="psS", bufs=2, space="PSUM") as psS,
            tc.tile_pool(name="psacc", bufs=4, space="PSUM") as psacc,
        ):
            # ---- constants -------------------------------------------------
            identt = constp.tile([128, 128], FP32R, tag="ident")
            nc.sync.dma_start(identt[:], id_d.ap())
            trit = constp.tile([128, 128], FP32, tag="tri")
            nc.sync.dma_start(trit[:], tri_d.ap())
            onest = constp.tile([128, 128], FP32R, tag="ones")
            nc.sync.dma_start(onest[:], one_d.ap())

            # ---- phase 0: weight slices, transposed on PE ------------------
            wqT = constp.tile([128, KO, 128], FP32R, tag="wqT")
            wkT = constp.tile([128, KO, 128], FP32R, tag="wkT")
            wvT = constp.tile([128, KO, 128], FP32R, tag="wvT")
            woT = constp.tile([128, KO, 128], FP32R, tag="woT")

            for dram, dst, natural in (
                (wq_d, wqT, True),
                (wk_d, wkT, True),
                (wv_d, wvT, True),
                (wo_d, woT, False),
            ):
                wl = xloadp.tile([128, KO, 128], FP32R, tag="wload")
                if natural:
                    nc.sync.dma_start(
                        wl[:], dram.ap().rearrange("c (ko p) -> c ko p", p=128)
                    )
                else:
                    nc.sync.dma_start(
                        wl[:], dram.ap().rearrange("(ko p) c -> p ko c", p=128)
                    )
                for g in range(2):
                    pst = psS.tile([128, 512], FP32R, tag="psS")
                    for j in range(4):
                        db = g * 4 + j
                        nc.tensor.transpose(
                            pst[:, j * 128 : (j + 1) * 128], wl[:, db, :], identt[:]
                        )
                    nc.vector.tensor_copy(
                        dst[:, g * 4 : (g + 1) * 4, :].rearrange("p a b -> p (a b)"),
                        pst[:],
                    )

            # ---- phase 1: x transpose + QKV projections + V1 assembly ------
            qT0 = bigp.tile([128, NST, ST], FP32R, tag="qT0")  # h0 rows, 64:128 zero
            qT1 = bigp.tile([128, NST, ST], FP32R, tag="qT1")  # h1 rows, 0:64 zero
            kT = bigp.tile([128, NST, ST], FP32R, tag="kT")
            vT = bigp.tile([128, NST, ST], FP32R, tag="vT")
            v1 = bigp.tile([128, B, 2, NKB, 65], FP32R, tag="v1")
            # ones column via DVE (a strided 4B-element DMA is pathological)
            nc.vector.tensor_copy(
                v1[:, :, :, :, 64],
                onest[:, 0 : B * 2 * NKB].rearrange("p (b h k) -> p b h k", b=B, h=2),
            )
            nc.sync.dma_start(
                qT0[64:128, :, :].rearrange("p a b -> p (a b)"), zero_d.ap()
            )
            nc.sync.dma_start(
                qT1[0:64, :, :].rearrange("p a b -> p (a b)"), zero_d.ap()
            )

            def v1_assemble(st):
                # V1 (k-major V + ones) for s-tile st's 4 k-blocks
                b = st // 4
                for hp in range(2):
                    pst = psS.tile([128, 512], FP32R, tag="psS", name=f"v1t_{st}_{hp}")
                    for j in range(4):
                        kb = (st % 4) * 4 + j
                        sti, off = divmod(b * S + kb * 128, ST)
                        nc.tensor.transpose(
                            pst[:, j * 64 : (j + 1) * 64],
                            vT[64 * hp : 64 * hp + 64, sti, off : off + 128],
                            identt[64 * hp : 64 * hp + 64, 64 * hp : 64 * hp + 64],
                        )
                    nc.vector.tensor_copy(
                        v1[:, b, hp, (st % 4) * 4 : (st % 4) * 4 + 4, 0:64],
                        pst[:, 0:256].rearrange("p (a c) -> p a c", a=4),
                    )

            for st in range(NST):
                xls = []
                for h in range(2):
                    xl = xloadp.tile([128, 2, D], FP32R, tag="xl")
                    r0 = st * ST + h * 256
                    nc.sync.dma_start(
                        xl[:],
                        x_d.ap()[r0 : r0 + 256, :].rearrange(
                            "(sb p) d -> p sb d", p=128
                        ),
                    )
                    xls.append(xl)
                xt = xtp.tile([128, KO, ST], FP32R, tag="xt")
                psq = psacc.tile([128, ST], FP32, tag="acc")
                psk = psacc.tile([128, ST], FP32, tag="acc")
                psv = psacc.tile([128, ST], FP32, tag="acc")
                for db in range(KO):
                    pst = psS.tile([128, 512], FP32R, tag="psS")
                    for sb in range(4):
                        nc.tensor.transpose(
                            pst[:, sb * 128 : (sb + 1) * 128],
                            xls[sb // 2][:, sb % 2, db * 128 : (db + 1) * 128],
                            identt[:],
                        )
                    nc.vector.tensor_copy(xt[:, db, :], pst[:])
                if st > 0:
                    v1_assemble(st - 1)
                for db in range(KO):
                    first, last = db == 0, db == KO - 1
                    nc.tensor.matmul(
                        psq[:], wqT[:, db, :], xt[:, db, :], start=first, stop=last
                    )
                    nc.tensor.matmul(
                        psk[:], wkT[:, db, :], xt[:, db, :], start=first, stop=last
                    )
                    nc.tensor.matmul(
                        psv[:], wvT[:, db, :], xt[:, db, :], start=first, stop=last
                    )
                nc.vector.tensor_copy(qT0[0:64, st, :], psq[0:64, :])
                nc.vector.tensor_copy(qT1[64:128, st, :], psq[64:128, :])
                nc.vector.tensor_copy(kT[:, st, :], psk[:])
                nc.vector.tensor_copy(vT[:, st, :], psv[:])
            v1_assemble(NST - 1)

            # ---- phase 3+4: attention + output projection per batch --------
            # Software pipelining: scores of group g+1 are issued before the
            # AV matmuls of group g (PE never waits on the ACT exp), and the
            # softmax-normalization tail of q-tile qt is issued during q-tile
            # qt+1 so its DMA round-trip latency is hidden.
            qTs = (qT0, qT1)
            rpads = {}
            deferred = []  # closures: division tails + outproj, drained lazily
            for b in range(B):
                ctx = bigp.tile([128, S], FP32R, tag=f"ctx{b}")

                def div_head(qt, hp, cd, b=b, ctx=ctx):
                    # normalization tail: needs cd (ctx+den copy) + its rpad
                    q0 = qt * QTW
                    rrow = workp.tile(
                        [1, QTW], FP32R, tag="rrow", name=f"rrow_{b}_{qt}_{hp}"
                    )
                    nc.gpsimd.dma_start(rrow[:], rpads[(b, qt, hp)][:])
                    rbc = psacc.tile(
                        [128, QTW], FP32, tag="acc", name=f"rbc_{b}_{qt}_{hp}"
                    )
                    nc.tensor.matmul(
                        rbc[:], onest[0:1, :], rrow[:], start=True, stop=True
                    )
                    nc.vector.tensor_tensor(
                        ctx[64 * hp : 64 * hp + 64, q0 : q0 + QTW],
                        cd[0:64, :],
                        rbc[0:64, :],
                        OP.mult,
                    )

                def outproj(sb, ot, b=b, ctx=ctx):
                    po = psacc.tile(
                        [128, 512], FP32, tag="acc", name=f"po_{b}_{sb}_{ot}"
                    )
                    nc.tensor.matmul(
                        po[:],
                        ctx[:, sb * 128 : (sb + 1) * 128],
                        woT[:, ot * 4 : (ot + 1) * 4, :].rearrange("p a b -> p (a b)"),
                        start=True,
                        stop=True,
                    )
                    ost = workp.tile([128, 512], FP32, tag="ost")
                    if (sb + ot) % 2 == 0:
                        nc.scalar.activation(ost[:], po[:], AF.Copy)
                    else:
                        nc.vector.tensor_copy(ost[:], po[:])
                    r0 = b * S + sb * 128
                    nc.sync.dma_start(
                        out_d.ap()[r0 : r0 + 128, ot * 512 : (ot + 1) * 512],
                        ost[:],
                    )

                for qt in range(NQT):
                    q0 = qt * QTW
                    stq = (b * S + q0) // ST
                    av0 = psacc.tile([65, QTW], FP32, tag="acc", name=f"av0_{b}_{qt}")
                    av1 = psacc.tile([65, QTW], FP32, tag="acc", name=f"av1_{b}_{qt}")
                    av = (av0, av1)
                    nkb = (q0 + QTW) // 128
                    groups = [
                        (hp, list(range(g, min(g + GRP, nkb))))
                        for hp in range(2)
                        for g in range(0, nkb, GRP)
                    ]
                    prev = None  # (hp, kbs, pt)
                    for gi, grp_item in enumerate(groups + [None]):
                        hp, kbs = grp_item if grp_item is not None else (None, None)
                        if gi < len(groups):
                            pss = psS.tile(
                                [128, GRP, QTW], FP32, tag="psS",
                                name=f"pss_{b}_{qt}_{hp}_{kbs[0]}",
                            )
                            for j, kb in enumerate(kbs):
                                k0 = kb * 128
                                stk, offk = divmod(b * S + k0, ST)
                                nc.tensor.matmul(
                                    pss[:, j, :],
                                    kT[:, stk, offk : offk + 128],
                                    qTs[hp][:, stq, :],
                                    start=True,
                                    stop=True,
                                )
                                if k0 >= q0:
                                    j0 = k0 - q0
                                    nc.vector.tensor_tensor(
                                        pss[:, j, j0 : j0 + 128],
                                        pss[:, j, j0 : j0 + 128],
                                        trit[:],
                                        OP.add,
                                    )
                            pt = ptp.tile([128, GRP, QTW], FP32R, tag="pt")
                            nc.scalar.activation(
                                pt[:, :, :].rearrange("p a b -> p (a b)"),
                                pss[:, :, :].rearrange("p a b -> p (a b)"),
                                AF.Exp,
                                scale=0.125,
                            )
                        # AV of the previous group (pipelined behind scores)
                        if prev is not None:
                            phl, pkbs, ppt = prev
                            for j, kb in enumerate(pkbs):
                                j0 = max(0, kb * 128 - q0)
                                nc.tensor.matmul(
                                    av[phl][:, j0:QTW],
                                    v1[:, b, phl, kb, :],
                                    ppt[:, j, j0:QTW],
                                    start=(kb == 0),
                                    stop=(kb == nkb - 1),
                                )
                        prev = (hp, kbs, pt) if gi < len(groups) else None
                        # drain deferred work (qt-1 tails) spread across groups
                        if gi >= 1 and deferred:
                            n_emit = 2 if len(deferred) > len(groups) - gi else 1
                            for _ in range(n_emit):
                                if deferred:
                                    deferred.pop(0)()
                    # start this q-tile's normalization chains (latency hidden)
                    for hp in range(2):
                        cd = cdp.tile(
                            [65, QTW], FP32R, tag="cd", name=f"cd_{b}_{qt}_{hp}"
                        )
                        nc.scalar.activation(cd[:], av[hp][:], AF.Copy)
                        dpad = dramp.tile([1, QTW], FP32, tag="dpad")
                        nc.sync.dma_start(dpad[:], cd.bitcast(FP32)[64:65, :])
                        denT = workp.tile([128, 4], FP32, tag="denT")
                        nc.sync.dma_start(
                            denT[:], dpad.rearrange("o (p j) -> p (o j)", p=128)
                        )
                        recT = workp.tile([128, 4], FP32, tag="recT")
                        nc.vector.reciprocal(recT[:], denT[:])
                        rpad = dramp.tile(
                            [1, QTW], FP32, tag="rpad", name=f"rpad_{b}_{qt}_{hp}"
                        )
                        nc.sync.dma_start(
                            rpad.rearrange("o (p j) -> p (o j)", p=128), recT[:]
                        )
                        rpads[(b, qt, hp)] = rpad
                        deferred.append(
                            lambda qt=qt, hp=hp, cd=cd, fn=div_head: fn(qt, hp, cd)
                        )
                    for sb in range(4 * qt, 4 * qt + 4):
                        for ot in range(2):
                            deferred.append(lambda sb=sb, ot=ot, fn=outproj: fn(sb, ot))
            for fn in deferred:
                fn()
    nc.compile()
    return nc


def _get_nc():
    if "nc" not in _CACHE:
        _CACHE["nc"] = _build()
    return _CACHE["nc"]


def _consts():
    ident = np.eye(128, dtype=np.float32)
    p = np.arange(128)
    tri = np.where(p[:, None] <= p[None, :], 0.0, NEG).astype(np.float32)
    ones = np.ones((128, 128), dtype=np.float32)
    zeros = np.zeros((64, BS), dtype=np.float32)
    return ident, tri, ones, zeros


def make_in_maps(inputs):
    x = np.ascontiguousarray(np.asarray(inputs["x"], dtype=np.float32)).reshape(BS, D)
    Wq = np.asarray(inputs["Wq"], dtype=np.float32)
    Wk = np.asarray(inputs["Wk"], dtype=np.float32)
    Wv = np.asarray(inputs["Wv"], dtype=np.float32)
    Wo = np.asarray(inputs["Wo"], dtype=np.float32)

    ident, tri, ones, zeros = _consts()
    in_maps = []
    for c in range(NCORES):
        sl = slice(c * CP, (c + 1) * CP)
        in_maps.append(
            {
                "x_in": x,
                "wq_in": np.ascontiguousarray(Wq[sl]),
                "wk_in": np.ascontiguousarray(Wk[sl]),
                "wv_in": np.ascontiguousarray(Wv[sl]),
                "wo_in": np.ascontiguousarray(Wo[:, sl]),
                "ident_in": ident,
                "tri_in": tri,
                "ones_in": ones,
                "zeros_in": zeros,
            }
        )
    return in_maps


def reduce_outputs(results, bo):
    acc = np.zeros((BS, D), dtype=np.float64)
    for r in results:
        acc += r["out_p"]
    acc += np.asarray(bo, dtype=np.float64)
    return acc.astype(np.float32).reshape(B, S, D)


def kernel(**inputs):
    bo = np.asarray(inputs["bo"], dtype=np.float32)
    in_maps = make_in_maps(inputs)
    nc = _get_nc()
    res = run_bass_kernel_spmd(nc, in_maps, core_ids=list(range(NCORES)))
    return reduce_outputs(res.results, bo)


# revision 29
# speedup vs baseline: 1.2716x; 1.1616x over previous
"""Causal multi-head attention (B=2, S=2048, D=1024, H=16) on 8 Trainium2
NeuronCores, tensor-parallel over heads: core c owns heads 2c and 2c+1
(a 128-wide slice of the QKV output dim / Wo input dim).

All matmuls float32r (TF32-like, full rate at N>=256). Key perf constraints
discovered on HW:
  - PE_HAM only counts full-array (K=128) matmuls as "busy"; K=64 streams
    run the PE at 1.2 GHz. So scores use K=128 with per-head zero-padded
    QT operands instead of K=64 contractions.
  - Mixing K=64 and K=128 matmuls breaks LDWEIGHTS pipelining (~2x); a
    uniform K=128 stream (M may vary) runs at ~227ns per N=512 matmul.
  - DVE reciprocal costs ~6.4 cyc per free-element per lane, so softmax
    denominators are DMA-reshaped to a [128, 4] partition-parallel layout,
    reciprocal'd there (~180ns), and DMA'd back for a K=1 broadcast matmul.

Dataflow per core:
  x --PE-transpose--> xT (per 512-row s-tile)
  QT0/QT1 (zero-padded per head), KT, VT = W.T @ xT  [c, 4096] layout
  per (batch, 512-q-tile, head): for k-blocks in groups of 2:
     S^T[k,q] = kT_blk.T @ QTh_tile   (K=128, N=512)
     P^T = exp(0.125*(S^T + tri_mask))   one ACT op per group, f32r out
     ctx^T[65,512] += V1_blk.T @ P^T[j0:]  (V1 = [V | 1]: row 64 = denom)
  denominators: DMA-> [128,4] -> DVE reciprocal -> DMA -> [1,512] row ->
     K=1 ones matmul broadcast -> DVE multiply (normalize ctx).
  out_partial = ctx^T.T @ WoT per 128-row s-block; PSUM->SBUF->DRAM.

Host side shards weights across cores and sums the 8 partial outputs + bias.
"""

import numpy as np

import concourse.mybir as mybir
from concourse import bacc
from concourse.bass_utils import run_bass_kernel_spmd
from concourse.tile import TileContext

B, S, D, H = 2, 2048, 1024, 16
HD = D // H              # 64
BS = B * S               # 4096
NCORES = 8
CP = 128                 # c-dim per core (2 heads x 64)
ST = 512                 # s-tile width for projections
NST = BS // ST           # 8
QTW = 512                # q-tile width for attention
NQT = S // QTW           # 4 per batch
KO = D // 128            # 8 d_in blocks
NKB = S // 128           # 16 k-blocks per batch
GRP = 2                  # k-blocks per exp group (2 PSUM banks)
NEG = -1.0e30

FP32 = mybir.dt.float32
FP32R = mybir.dt.float32r
AF = mybir.ActivationFunctionType
OP = mybir.AluOpType

_CACHE = {}


def _build():
    nc = bacc.Bacc("TRN2", target_bir_lowering=False, debug=False, num_devices=NCORES)

    # All host-side-pretransposed during shard prep: xT = x.T, w*T = W[sl].T,
    # woT = Wo[:, sl].T.
    x_d = nc.dram_tensor("xT_in", [D, BS], FP32R, kind="ExternalInput")
    wq_d = nc.dram_tensor("wqT_in", [D, CP], FP32R, kind="ExternalInput")
    wk_d = nc.dram_tensor("wkT_in", [D, CP], FP32R, kind="ExternalInput")
    wv_d = nc.dram_tensor("wvT_in", [D, CP], FP32R, kind="ExternalInput")
    wo_d = nc.dram_tensor("woT_in", [CP, D], FP32R, kind="ExternalInput")
    id_d = nc.dram_tensor("ident_in", [128, 128], FP32R, kind="ExternalInput")
    tri_d = nc.dram_tensor("tri_in", [128, 128], FP32, kind="ExternalInput")
    one_d = nc.dram_tensor("ones_in", [128, 128], FP32R, kind="ExternalInput")
    zero_d = nc.dram_tensor("zeros_in", [64, BS], FP32R, kind="ExternalInput")
    out_d = nc.dram_tensor("out_p", [BS, D], FP32, kind="ExternalOutput")

    with TileContext(nc) as tc:
        with (
            tc.tile_pool(name="const", bufs=1) as constp,
            tc.tile_pool(name="big", bufs=1) as bigp,
            tc.tile_pool(name="xt", bufs=3) as xtp,
            tc.tile_pool(name="pt", bufs=2) as ptp,
            tc.tile_pool(name="work", bufs=3) as workp,
            tc.tile_pool(name="cdp", bufs=4) as cdp,
            tc.tile_pool(name="dram", bufs=8, space="DRAM") as dramp,
            tc.tile_pool(name="psS", bufs=2, space="PSUM") as psS,
            tc.tile_pool(name="psacc", bufs=4, space="PSUM") as psacc,
        ):
            # ---- constants -------------------------------------------------
            identt = constp.tile([128, 128], FP32R, tag="ident")
            nc.sync.dma_start(identt[:], id_d.ap())
            trit = constp.tile([128, 128], FP32, tag="tri")
            nc.sync.dma_start(trit[:], tri_d.ap())
            onest = constp.tile([128, 128], FP32R, tag="ones")
            nc.sync.dma_start(onest[:], one_d.ap())

            # ---- phase 0: weight slices (already transposed on host) -------
            wqT = constp.tile([128, KO, 128], FP32R, tag="wqT")
            wkT = constp.tile([128, KO, 128], FP32R, tag="wkT")
            wvT = constp.tile([128, KO, 128], FP32R, tag="wvT")
            woT = constp.tile([128, KO, 128], FP32R, tag="woT")
            for dram, dst in ((wq_d, wqT), (wk_d, wkT), (wv_d, wvT)):
                nc.sync.dma_start(
                    dst[:], dram.ap().rearrange("(ko p) c -> p ko c", p=128)
                )
            nc.sync.dma_start(
                woT[:], wo_d.ap().rearrange("c (ko p) -> c ko p", p=128)
            )

            # ---- phase 1: x transpose + QKV projections + V1 assembly ------
            qT0 = bigp.tile([128, NST, ST], FP32R, tag="qT0")  # h0 rows, 64:128 zero
            qT1 = bigp.tile([128, NST, ST], FP32R, tag="qT1")  # h1 rows, 0:64 zero
            kT = bigp.tile([128, NST, ST], FP32R, tag="kT")
            vT = bigp.tile([128, NST, ST], FP32R, tag="vT")
            v1 = bigp.tile([128, B, 2, NKB, 65], FP32R, tag="v1")
            # ones column via DVE (a strided 4B-element DMA is pathological)
            nc.vector.tensor_copy(
                v1[:, :, :, :, 64],
                onest[:, 0 : B * 2 * NKB].rearrange("p (b h k) -> p b h k", b=B, h=2),
            )
            nc.sync.dma_start(
                qT0[64:128, :, :].rearrange("p a b -> p (a b)"), zero_d.ap()
            )
            nc.sync.dma_start(
                qT1[0:64, :, :].rearrange("p a b -> p (a b)"), zero_d.ap()
            )

            def v1_assemble(st):
                # V1 (k-major V + ones) for s-tile st's 4 k-blocks
                b = st // 4
                for hp in range(2):
                    pst = psS.tile([128, 512], FP32R, tag="psS", name=f"v1t_{st}_{hp}")
                    for j in range(4):
                        kb = (st % 4) * 4 + j
                        sti, off = divmod(b * S + kb * 128, ST)
                        nc.tensor.transpose(
                            pst[:, j * 64 : (j + 1) * 64],
                            vT[64 * hp : 64 * hp + 64, sti, off : off + 128],
                            identt[64 * hp : 64 * hp + 64, 64 * hp : 64 * hp + 64],
                        )
                    nc.vector.tensor_copy(
                        v1[:, b, hp, (st % 4) * 4 : (st % 4) * 4 + 4, 0:64],
                        pst[:, 0:256].rearrange("p (a c) -> p a c", a=4),
                    )

            for st in range(NST):
                xt = xtp.tile([128, KO, ST], FP32R, tag="xt")
                nc.sync.dma_start(
                    xt[:],
                    x_d.ap()[:, st * ST : (st + 1) * ST].rearrange(
                        "(ko p) s -> p ko s", p=128
                    ),
                )
                psq = psacc.tile([128, ST], FP32, tag="acc")
                psk = psacc.tile([128, ST], FP32, tag="acc")
                psv = psacc.tile([128, ST], FP32, tag="acc")
                if st > 0:
                    v1_assemble(st - 1)
                for db in range(KO):
                    first, last = db == 0, db == KO - 1
                    nc.tensor.matmul(
                        psq[:], wqT[:, db, :], xt[:, db, :], start=first, stop=last
                    )
                    nc.tensor.matmul(
                        psk[:], wkT[:, db, :], xt[:, db, :], start=first, stop=last
                    )
                    nc.tensor.matmul(
                        psv[:], wvT[:, db, :], xt[:, db, :], start=first, stop=last
                    )
                nc.vector.tensor_copy(qT0[0:64, st, :], psq[0:64, :])
                nc.vector.tensor_copy(qT1[64:128, st, :], psq[64:128, :])
                nc.vector.tensor_copy(kT[:, st, :], psk[:])
                nc.vector.tensor_copy(vT[:, st, :], psv[:])
            v1_assemble(NST - 1)

            # ---- phase 3+4: attention + output projection per batch --------
            # Software pipelining: scores of group g+1 are issued before the
            # AV matmuls of group g (PE never waits on the ACT exp), and the
            # softmax-normalization tail of q-tile qt is issued during q-tile
            # qt+1 so its DMA round-trip latency is hidden.
            qTs = (qT0, qT1)
            rpads = {}
            deferred = []  # closures: division tails + outproj, drained lazily
            for b in range(B):
                ctx = bigp.tile([128, S], FP32R, tag=f"ctx{b}")

                def div_head(qt, hp, cd, b=b, ctx=ctx):
                    # normalization tail: needs cd (ctx+den copy) + its rpad
                    q0 = qt * QTW
                    rrow = workp.tile(
                        [1, QTW], FP32R, tag="rrow", name=f"rrow_{b}_{qt}_{hp}"
                    )
                    nc.gpsimd.dma_start(rrow[:], rpads[(b, qt, hp)][:])
                    rbc = psacc.tile(
                        [128, QTW], FP32, tag="acc", name=f"rbc_{b}_{qt}_{hp}"
                    )
                    nc.tensor.matmul(
                        rbc[:], onest[0:1, :], rrow[:], start=True, stop=True
                    )
                    nc.vector.tensor_tensor(
                        ctx[64 * hp : 64 * hp + 64, q0 : q0 + QTW],
                        cd[0:64, :],
                        rbc[0:64, :],
                        OP.mult,
                    )

                def outproj(sb, ot, b=b, ctx=ctx):
                    po = psacc.tile(
                        [128, 512], FP32, tag="acc", name=f"po_{b}_{sb}_{ot}"
                    )
                    nc.tensor.matmul(
                        po[:],
                        ctx[:, sb * 128 : (sb + 1) * 128],
                        woT[:, ot * 4 : (ot + 1) * 4, :].rearrange("p a b -> p (a b)"),
                        start=True,
                        stop=True,
                    )
                    ost = workp.tile([128, 512], FP32, tag="ost")
                    if (sb + ot) % 2 == 0:
                        nc.scalar.activation(ost[:], po[:], AF.Copy)
                    else:
                        nc.vector.tensor_copy(ost[:], po[:])
                    r0 = b * S + sb * 128
                    nc.sync.dma_start(
                        out_d.ap()[r0 : r0 + 128, ot * 512 : (ot + 1) * 512],
                        ost[:],
                    )

                for qt in range(NQT):
                    q0 = qt * QTW
                    stq = (b * S + q0) // ST
                    av0 = psacc.tile([65, QTW], FP32, tag="acc", name=f"av0_{b}_{qt}")
                    av1 = psacc.tile([65, QTW], FP32, tag="acc", name=f"av1_{b}_{qt}")
                    av = (av0, av1)
                    nkb = (q0 + QTW) // 128
                    groups = [
                        (hp, list(range(g, min(g + GRP, nkb))))
                        for hp in range(2)
                        for g in range(0, nkb, GRP)
                    ]
                    prev = None  # (hp, kbs, pt)
                    for gi, grp_item in enumerate(groups + [None]):
                        hp, kbs = grp_item if grp_item is not None else (None, None)
                        if gi < len(groups):
                            pss = psS.tile(
                                [128, GRP, QTW], FP32, tag="psS",
                                name=f"pss_{b}_{qt}_{hp}_{kbs[0]}",
                            )
                            for j, kb in enumerate(kbs):
                                k0 = kb * 128
                                stk, offk = divmod(b * S + k0, ST)
                                nc.tensor.matmul(
                                    pss[:, j, :],
                                    kT[:, stk, offk : offk + 128],
                                    qTs[hp][:, stq, :],
                                    start=True,
                                    stop=True,
                                )
                                if k0 >= q0:
                                    j0 = k0 - q0
                                    nc.vector.tensor_tensor(
                                        pss[:, j, j0 : j0 + 128],
                                        pss[:, j, j0 : j0 + 128],
                                        trit[:],
                                        OP.add,
                                    )
                            pt = ptp.tile([128, GRP, QTW], FP32R, tag="pt")
                            nc.scalar.activation(
                                pt[:, :, :].rearrange("p a b -> p (a b)"),
                                pss[:, :, :].rearrange("p a b -> p (a b)"),
                                AF.Exp,
                                scale=0.125,
                            )
                        # AV of the previous group (pipelined behind scores)
                        if prev is not None:
                            phl, pkbs, ppt = prev
                            for j, kb in enumerate(pkbs):
                                j0 = max(0, kb * 128 - q0)
                                nc.tensor.matmul(
                                    av[phl][:, j0:QTW],
                                    v1[:, b, phl, kb, :],
                                    ppt[:, j, j0:QTW],
                                    start=(kb == 0),
                                    stop=(kb == nkb - 1),
                                )
                        prev = (hp, kbs, pt) if gi < len(groups) else None
                        # drain deferred work (qt-1 tails) spread across groups
                        if gi >= 1 and deferred:
                            n_emit = 2 if len(deferred) > len(groups) - gi else 1
                            for _ in range(n_emit):
                                if deferred:
                                    deferred.pop(0)()
                    # start this q-tile's normalization chains (latency hidden)
                    for hp in range(2):
                        cd = cdp.tile(
                            [65, QTW], FP32R, tag="cd", name=f"cd_{b}_{qt}_{hp}"
                        )
                        nc.scalar.activation(cd[:], av[hp][:], AF.Copy)
                        dpad = dramp.tile([1, QTW], FP32, tag="dpad")
                        nc.sync.dma_start(dpad[:], cd.bitcast(FP32)[64:65, :])
                        denT = workp.tile([128, 4], FP32, tag="denT")
                        nc.sync.dma_start(
                            denT[:], dpad.rearrange("o (p j) -> p (o j)", p=128)
                        )
                        recT = workp.tile([128, 4], FP32, tag="recT")
                        nc.vector.reciprocal(recT[:], denT[:])
                        rpad = dramp.tile(
                            [1, QTW], FP32, tag="rpad", name=f"rpad_{b}_{qt}_{hp}"
                        )
                        nc.sync.dma_start(
                            rpad.rearrange("o (p j) -> p (o j)", p=128), recT[:]
                        )
                        rpads[(b, qt, hp)] = rpad
                        deferred.append(
                            lambda qt=qt, hp=hp, cd=cd, fn=div_head: fn(qt, hp, cd)
                        )
                    for sb in range(4 * qt, 4 * qt + 4):
                        for ot in range(2):
                            deferred.append(lambda sb=sb, ot=ot, fn=outproj: fn(sb, ot))
            for fn in deferred:
                fn()
    nc.compile()
    return nc


def _get_nc():
    if "nc" not in _CACHE:
        _CACHE["nc"] = _build()
    return _CACHE["nc"]


def _consts():
    ident = np.eye(128, dtype=np.float32)
    p = np.arange(128)
    tri = np.where(p[:, None] <= p[None, :], 0.0, NEG).astype(np.float32)
    ones = np.ones((128, 128), dtype=np.float32)
    zeros = np.zeros((64, BS), dtype=np.float32)
    return ident, tri, ones, zeros


def make_in_maps(inputs):
    x = np.asarray(inputs["x"], dtype=np.float32).reshape(BS, D)
    xT = np.ascontiguousarray(x.T)
    Wq = np.asarray(inputs["Wq"], dtype=np.float32)
    Wk = np.asarray(inputs["Wk"], dtype=np.float32)
    Wv = np.asarray(inputs["Wv"], dtype=np.float32)
    Wo = np.asarray(inputs["Wo"], dtype=np.float32)

    ident, tri, ones, zeros = _consts()
    in_maps = []
    for c in range(NCORES):
        sl = slice(c * CP, (c + 1) * CP)
        in_maps.append(
            {
                "xT_in": xT,
                "wqT_in": np.ascontiguousarray(Wq[sl].T),
                "wkT_in": np.ascontiguousarray(Wk[sl].T),
                "wvT_in": np.ascontiguousarray(Wv[sl].T),
                "woT_in": np.ascontiguousarray(Wo[:, sl].T),
                "ident_in": ident,
                "tri_in": tri,
                "ones_in": ones,
                "zeros_in": zeros,
            }
        )
    return in_maps


def reduce_outputs(results, bo):
    acc = np.zeros((BS, D), dtype=np.float64)
    for r in results:
        acc += r["out_p"]
    acc += np.asarray(bo, dtype=np.float64)
    return acc.astype(np.float32).reshape(B, S, D)


def kernel(**inputs):
    bo = np.asarray(inputs["bo"], dtype=np.float32)
    in_maps = make_in_maps(inputs)
    nc = _get_nc()
    res = run_bass_kernel_spmd(nc, in_maps, core_ids=list(range(NCORES)))
    return reduce_outputs(res.results, bo)
